# revision 22
# baseline (speedup 1.0000x reference)
"""AdaIN statistics kernel for TRN2, SPMD across 8 NeuronCores.

Input : f_vol [32, 512, 64, 64] f32
Output: [32, 1024] f32 = concat([mean over (h,w), unbiased std over (h,w)], axis=-1)

Sharding: data-parallel over batch — each of the 8 cores handles 4 batches
([4, 512, 64, 64] shard, 32 MiB). No collectives; the host concatenates the
8 per-core [4, 1024] outputs.

Per core: 2048 rows (b*512+c) x 4096 spatial elems, streamed through SBUF
on the SP HWDGE queue as 7 m=2 slabs (32 KiB/partition descriptors, HBM-cap
rate) + 2 m=1 tail slabs (16 KiB descriptors).

v2 rebalance vs the all-DVE baseline (which had DVE busy ~92 us ~= the whole
stream time, so on slow-HBM runs bn_stats lagged the last DMA byte by 15+ us):
  - ACT consumes m=2 slabs 1 and 3 via Copy/Square+accumulate and also row 1
    of slab 6 (the last m=2 slab, which lands ~12 us before stream end —
    row-splitting it across DVE+ACT halves its post-arrival latency).
  - DVE consumes slabs 0, 2, 4, 5, row 0 of slab 6, and the two m=1 tails,
    ~68 us of bn_stats/bn_aggr against a ~95-100 us stream: enough slack
    that DVE is always idle-ready when the tail slabs land.
  - ACT's one-time ACT_TABLE_LOAD (~1.5 us) is triggered by a dummy copy at
    block start instead of inside the first epilogue.
  - The final F->T transpose is gated on exactly the tail epilogues (the
    only F writers), not on every slab's epilogue.

Outputs: m=2 slabs DMA their [128, 2, 2] stats directly (interleaved channel
layout, 3D AP; the ~7 us completion latency of the tiny per-partition
descriptors hides mid-stream). The two m=1 tails write stat columns of
F[128, 4]; DVE block-transposes F -> T so the one final DMA writes 4
contiguous 512 B runs.

Raw Bass with manual semaphores; every cross-instruction data edge is
covered by an explicit semaphore observation for the CoreSim race detector.
SWDGE requires sem values to start at 0, so semaphores are never reused.
"""

from contextlib import ExitStack

import numpy as np

B, C, H, W = 32, 512, 64, 64
N_CORES = 8
B_LOCAL = B // N_CORES  # 4
N = H * W  # 4096
P = 128
ROWS = B_LOCAL * C  # 2048

NBUF = 5  # xt ring slots (32 KiB/partition each)
NRES = 4  # res ring slots / out-DMA sems

M2SLABS = 7  # m=2 slabs, rows 0..1791
ACT_FULL = (1, 3)  # m=2 slabs fully consumed by ACT accumulate
SPLIT = 6  # last m=2 slab: DVE row 0, ACT row 1
DVE_FULL = (0, 2, 4, 5)
TAILROWS = 2  # m=1 tail slabs, rows 1792..2047
# tail row 1 column split: A-part groups (8 bn_stats over 3968 cols) + one
# trailing 128-col B-part (512 B/partition descriptors, the line-rate floor)
T1GROUPS = [512] * 7 + [384]
BCOL = sum(T1GROUPS)  # 3968

_CACHE = {}


def _build():
    import concourse.bass as bass
    from concourse import mybir

    nc = bass.Bass()
    x_ext = nc.declare_dram_parameter(
        "f_vol", [B_LOCAL, C, H, W], mybir.dt.float32, isOutput=False
    )
    out_ext = nc.declare_dram_parameter(
        "out", [B_LOCAL, 2 * C], mybir.dt.float32, isOutput=True
    )

    x = x_ext.ap().rearrange("b c h w -> (b c) (h w)")  # [2048, 4096]

    base_rows = [256 * j for j in range(M2SLABS)]
    tail_base = 256 * M2SLABS  # 1792
    tb, tc0 = divmod(tail_base, C)  # batch 3, channel 256
    for j in range(M2SLABS):
        assert (base_rows[j] % C) + 256 <= C, f"slab {j} crosses a batch"
    assert tc0 + TAILROWS * P <= C

    # --- semaphore value accounting (matches emission order below) ---
    # dve_stats: +1 per bn_stats. DVE order: slabs 0,2,4,5 (16 each),
    # split-row0 (8), tail0 (8), tail1 (8).
    dve_cum = {0: 16, 2: 32, 4: 48, 5: 64, "s6r0": 72, "t0": 80, "t1": 89}
    # mv_ready: +1 per bn_aggr, same order (2 per full slab, 1 each after).
    mv_after = {0: 2, 2: 4, 4: 6, 5: 8, "s6r0": 9, "t0": 10, "t1": 11}
    # act_stats: +1 per ACT accumulate pass: slab1 (4), slab3 (4), s6r1 (2).
    acts_after = {1: 4, 3: 8, "s6r1": 10}

    # res ring over the 7 per-slab out-DMAs
    rslot = {j: j % NRES for j in range(M2SLABS)}
    out_total = {s: 0 for s in range(NRES)}
    res_wait = {}
    for j in range(M2SLABS):
        s = rslot[j]
        res_wait[j] = out_total[s]
        out_total[s] += 16

    # act_done: +1 per ACT epilogue op, in ACT emission order:
    # epi0 (4), epi1 (8 acc-form ops), epi2, epi3, epi4, epi5, epi6
    # (2 mv-form + 4 acc-form), tail epis (2 each).
    actd_after = {}
    cact = 0
    for j in range(M2SLABS):
        if j in ACT_FULL:
            cact += 8
        elif j == SPLIT:
            cact += 6
        else:
            cact += 4
        actd_after[j] = cact
    for t in range(TAILROWS):
        cact += 2
        actd_after[("tail", t)] = cact
    ACT_TOTAL = cact

    with ExitStack() as ctx:
        block = ctx.enter_context(nc.Block(no_gpsimd_drain=True))
        dma_m2 = [
            ctx.enter_context(nc.semaphore(f"dma_m2_{j}")) for j in range(M2SLABS)
        ]
        dma_t = [ctx.enter_context(nc.semaphore(f"dma_t{t}")) for t in range(2)]
        dma_t1b = ctx.enter_context(nc.semaphore("dma_t1b"))
        dma_out = [ctx.enter_context(nc.semaphore(f"dma_out{s}")) for s in range(NRES)]
        dma_fin = ctx.enter_context(nc.semaphore("dma_fin"))
        dve_stats = ctx.enter_context(nc.semaphore("dve_stats"))
        act_stats = ctx.enter_context(nc.semaphore("act_stats"))
        mv_ready = ctx.enter_context(nc.semaphore("mv_ready"))
        act_done = ctx.enter_context(nc.semaphore("act_done"))
        trans_done = ctx.enter_context(nc.semaphore("trans_done"))

        xt = ctx.enter_context(nc.sbuf_tensor("xt", [P, NBUF, 2 * N], mybir.dt.float32))
        xt2 = ctx.enter_context(nc.sbuf_tensor("xt2", [P, 2, N], mybir.dt.float32))
        # stats slots: 0..3 = full DVE slabs 0,2,4,5; statsT for s6r0+tails
        stats = ctx.enter_context(
            nc.sbuf_tensor("stats", [P, 4, 2, 8, 6], mybir.dt.float32)
        )
        statsT = ctx.enter_context(
            nc.sbuf_tensor("statsT", [P, 3, 9, 6], mybir.dt.float32)
        )
        mv = ctx.enter_context(nc.sbuf_tensor("mv", [P, 4, 2, 2], mybir.dt.float32))
        mvT = ctx.enter_context(nc.sbuf_tensor("mvT", [P, 3, 2], mybir.dt.float32))
        res = ctx.enter_context(nc.sbuf_tensor("res", [P, NRES, 2, 2], mybir.dt.float32))
        F = ctx.enter_context(nc.sbuf_tensor("F", [P, 32], mybir.dt.float32))
        T = ctx.enter_context(nc.sbuf_tensor("T", [P, P], mybir.dt.float32))
        # ACT accumulators [sum, sumsq, tmp]: rows (1,r0),(1,r1),(3,r0),(3,r1),(6,r1)
        acc = ctx.enter_context(nc.sbuf_tensor("acc", [P, 5, 3], mybir.dt.float32))
        warm = ctx.enter_context(nc.sbuf_tensor("warm", [P, 2], mybir.dt.float32))

        sslot = {0: 0, 2: 1, 4: 2, 5: 3}  # stats/mv slot per full DVE slab
        accrow = {(1, 0): 0, (1, 1): 1, (3, 0): 2, (3, 1): 3, (6, 1): 4}

        @block.sync
        def _(sync):
            for j in range(M2SLABS):
                if j >= NBUF:
                    jp = j - NBUF
                    if jp in ACT_FULL:
                        sync.wait_ge(act_stats, acts_after[jp])
                    else:
                        sync.wait_ge(dve_stats, dve_cum[jp])
                    sync.wait_ge(dma_m2[jp], 16)
                src = x[base_rows[j] : base_rows[j] + 256, :].rearrange(
                    "(p m) f -> p (m f)", m=2
                )
                sync.dma_start(out=xt[:, j % NBUF, :], in_=src).then_inc(
                    dma_m2[j], 16
                )
            # tail 0: one m=1 slab. tail 1 (the last-arriving data) is split
            # by column so only a 512-col B-part trails the stream — the
            # exposed tail compute is then one bn_stats instead of eight.
            src = x[tail_base : tail_base + P, :]
            sync.dma_start(out=xt2[:, 0, :], in_=src).then_inc(dma_t[0], 16)
            srcA = x[tail_base + P : tail_base + 2 * P, 0:BCOL]
            sync.dma_start(out=xt2[:, 1, 0:BCOL], in_=srcA).then_inc(dma_t[1], 16)
            srcB = x[tail_base + P : tail_base + 2 * P, BCOL:N]
            sync.dma_start(out=xt2[:, 1, BCOL:N], in_=srcB).then_inc(dma_t1b, 16)

            # final output DMA: T rows 0..3 -> 4 contiguous 512 B runs
            sync.wait_ge(trans_done, 2 + P // 32)
            dst = bass.AP(
                tensor=out_ext,
                offset=tb * 2 * C + tc0,
                ap=[[P, TAILROWS], [C, 2], [1, P]],
            )
            sync.dma_start(out=dst, in_=T[0 : 2 * TAILROWS, 0:P]).then_inc(
                dma_fin, 16
            )
            for s in range(NRES):
                if out_total[s]:
                    sync.wait_ge(dma_out[s], out_total[s])
            sync.wait_ge(dma_fin, 16)

        @block.vector
        def _(vector):
            # trans_done: 1 = warm memset, 2 = F memset, 3..6 = transposes
            vector.memset(warm[:, :], 0.0).then_inc(trans_done, 1)
            vector.memset(F[:, :], 0.0).then_inc(trans_done, 1)

            for j in DVE_FULL:
                s = sslot[j]
                vector.wait_ge(dma_m2[j], 16)
                for r in range(2):
                    for g in range(8):
                        vector.bn_stats(
                            out=stats[:, s, r, g, :],
                            in_=xt[:, j % NBUF, (r * 8 + g) * 512 : (r * 8 + g + 1) * 512],
                        ).then_inc(dve_stats, 1)
                vector.wait_ge(dve_stats, dve_cum[j])
                for r in range(2):
                    vector.bn_aggr(
                        out=mv[:, s, r, :], in_=stats[:, s, r, :, :]
                    ).then_inc(mv_ready, 1)

            # split slab: row 0 only (row 1 is ACT's)
            vector.wait_ge(dma_m2[SPLIT], 16)
            for g in range(8):
                vector.bn_stats(
                    out=statsT[:, 0, g, :],
                    in_=xt[:, SPLIT % NBUF, g * 512 : (g + 1) * 512],
                ).then_inc(dve_stats, 1)
            vector.wait_ge(dve_stats, dve_cum["s6r0"])
            vector.bn_aggr(out=mvT[:, 0, :], in_=statsT[:, 0, 0:8, :]).then_inc(
                mv_ready, 1
            )

            for t in range(TAILROWS):
                vector.wait_ge(dma_t[t], 16)
                if t == 0:
                    for g in range(8):
                        vector.bn_stats(
                            out=statsT[:, 1, g, :],
                            in_=xt2[:, 0, g * 512 : (g + 1) * 512],
                        ).then_inc(dve_stats, 1)
                else:
                    c0 = 0
                    for g, gw in enumerate(T1GROUPS):
                        vector.bn_stats(
                            out=statsT[:, 2, g, :], in_=xt2[:, 1, c0 : c0 + gw]
                        ).then_inc(dve_stats, 1)
                        c0 += gw
                    assert c0 == BCOL
                    vector.wait_ge(dma_t1b, 16)
                    vector.bn_stats(
                        out=statsT[:, 2, len(T1GROUPS), :], in_=xt2[:, 1, BCOL:N]
                    ).then_inc(dve_stats, 1)
                vector.wait_ge(dve_stats, dve_cum[f"t{t}"])
                vector.bn_aggr(
                    out=mvT[:, 1 + t, :],
                    in_=statsT[:, 1 + t, 0 : (8 if t == 0 else len(T1GROUPS) + 1), :],
                ).then_inc(mv_ready, 1)

            # all F writers done -> 32x32 block transposes of F into T
            vector.wait_ge(act_done, ACT_TOTAL)
            vector.wait_ge(trans_done, 2)  # observe the F memset (same engine)
            for blk in range(P // 32):
                vector.transpose(
                    out=T[0:32, blk * 32 : blk * 32 + 32],
                    in_=F[blk * 32 : blk * 32 + 32, 0:32],
                ).then_inc(trans_done, 1)

        @block.scalar
        def _(scalar):
            A = 1.0 / np.sqrt(float(N) * (N - 1))
            cact = 0
            casts = 0

            # warm the ACT function table while the first slab streams
            scalar.wait_ge(trans_done, 1)
            scalar.activation(
                out=warm[:, 0:1],
                in_=warm[:, 1:2],
                func=mybir.ActivationFunctionType.Copy,
            )

            def acc_pass(j, rows):
                nonlocal casts
                scalar.wait_ge(dma_m2[j], 16)
                for r in rows:
                    row = xt[:, j % NBUF, r * N : (r + 1) * N]
                    ar = accrow[(j, r)]
                    scalar.activation(
                        out=row,
                        in_=row,
                        func=mybir.ActivationFunctionType.Copy,
                        accum_out=acc[:, ar, 0:1],
                    ).then_inc(act_stats, 1)
                    casts += 1
                    # observe the Copy (xt write + acc[0]) before Square
                    scalar.wait_ge(act_stats, casts)
                    scalar.activation(
                        out=row,
                        in_=row,
                        func=mybir.ActivationFunctionType.Square,
                        accum_out=acc[:, ar, 1:2],
                    ).then_inc(act_stats, 1)
                    casts += 1

            def out_dma(j):
                b, c0 = divmod(base_rows[j], C)
                s = rslot[j]
                dst = bass.AP(
                    tensor=out_ext,
                    offset=b * 2 * C + c0,
                    ap=[[2, P], [C, 2], [1, 2]],
                )
                scalar.dma_start(out=dst, in_=res[:, s, :, :]).then_inc(
                    dma_out[s], 16
                )

            def mv_stat_ops(mean_src, var_src, mean_dst, std_dst):
                nonlocal cact
                scalar.copy(out=mean_dst, in_=mean_src).then_inc(act_done, 1)
                scalar.activation(
                    out=std_dst,
                    in_=var_src,
                    func=mybir.ActivationFunctionType.Sqrt,
                    scale=float(N) / (N - 1),
                ).then_inc(act_done, 1)
                cact += 2

            def acc_stat_ops(ar, mean_dst, std_dst):
                nonlocal cact
                # mean = sum / N
                scalar.activation(
                    out=mean_dst,
                    in_=acc[:, ar, 0:1],
                    func=mybir.ActivationFunctionType.Copy,
                    scale=1.0 / N,
                ).then_inc(act_done, 1)
                # tmp = (sum*A)^2 = sum^2/(N(N-1))
                scalar.activation(
                    out=acc[:, ar, 2:3],
                    in_=acc[:, ar, 0:1],
                    func=mybir.ActivationFunctionType.Square,
                    scale=A,
                ).then_inc(act_done, 1)
                cact += 2
                scalar.wait_ge(act_done, cact)
                scalar.activation(
                    out=acc[:, ar, 2:3],
                    in_=acc[:, ar, 2:3],
                    func=mybir.ActivationFunctionType.Copy,
                    scale=-1.0,
                ).then_inc(act_done, 1)
                cact += 1
                scalar.wait_ge(act_done, cact)
                # std = sqrt(sumsq/(N-1) - sum^2/(N(N-1)))
                scalar.activation(
                    out=std_dst,
                    in_=acc[:, ar, 1:2],
                    func=mybir.ActivationFunctionType.Sqrt,
                    scale=1.0 / (N - 1),
                    bias=acc[:, ar, 2:3],
                ).then_inc(act_done, 1)
                cact += 1

            def epilogue(j):
                s = rslot[j]
                if j in ACT_FULL:
                    scalar.wait_ge(act_stats, acts_after[j])
                elif j == SPLIT:
                    scalar.wait_ge(mv_ready, mv_after["s6r0"])
                    scalar.wait_ge(act_stats, acts_after["s6r1"])
                else:
                    scalar.wait_ge(mv_ready, mv_after[j])
                if res_wait[j]:
                    scalar.wait_ge(dma_out[s], res_wait[j])
                for r in range(2):
                    mean_dst = res[:, s, 0, r : r + 1]
                    std_dst = res[:, s, 1, r : r + 1]
                    if j in ACT_FULL:
                        acc_stat_ops(accrow[(j, r)], mean_dst, std_dst)
                    elif j == SPLIT and r == 1:
                        acc_stat_ops(accrow[(j, 1)], mean_dst, std_dst)
                    elif j == SPLIT:
                        mv_stat_ops(
                            mvT[:, 0, 0:1], mvT[:, 0, 1:2], mean_dst, std_dst
                        )
                    else:
                        mv_stat_ops(
                            mv[:, sslot[j], r, 0:1],
                            mv[:, sslot[j], r, 1:2],
                            mean_dst,
                            std_dst,
                        )
                scalar.wait_ge(act_done, cact)  # res RAW before out DMA
                out_dma(j)
                assert cact == actd_after[j], (j, cact, actd_after[j])

            acc_pass(1, (0, 1))
            epilogue(0)
            epilogue(1)
            acc_pass(3, (0, 1))
            epilogue(2)
            epilogue(3)
            acc_pass(SPLIT, (1,))
            epilogue(4)
            epilogue(5)
            epilogue(SPLIT)

            # tail epilogues into F — kept last so ACT is idle-ready
            scalar.wait_ge(trans_done, 2)  # F memset observed
            for t in range(TAILROWS):
                scalar.wait_ge(mv_ready, mv_after[f"t{t}"])
                mv_stat_ops(
                    mvT[:, 1 + t, 0:1],
                    mvT[:, 1 + t, 1:2],
                    F[:, 2 * t : 2 * t + 1],
                    F[:, 2 * t + 1 : 2 * t + 2],
                )
                assert cact == actd_after[("tail", t)]
            assert cact == ACT_TOTAL, (cact, ACT_TOTAL)

    return nc


def kernel(f_vol: np.ndarray) -> np.ndarray:
    from concourse.bass_utils import run_bass_kernel_spmd

    if "nc" not in _CACHE:
        _CACHE["nc"] = _build()
    nc = _CACHE["nc"]

    f_vol = np.ascontiguousarray(f_vol, dtype=np.float32)
    in_maps = [
        {"f_vol": f_vol[i * B_LOCAL : (i + 1) * B_LOCAL]} for i in range(N_CORES)
    ]
    res = run_bass_kernel_spmd(nc, in_maps, core_ids=list(range(N_CORES)))
    return np.concatenate([res.results[i]["out"] for i in range(N_CORES)], axis=0)


# revision 23
# speedup vs baseline: 1.0263x; 1.0263x over previous
"""AdaIN statistics kernel for TRN2, SPMD across 8 NeuronCores.

Input : f_vol [32, 512, 64, 64] f32
Output: [32, 1024] f32 = concat([mean over (h,w), unbiased std over (h,w)], axis=-1)

Sharding: data-parallel over batch — each of the 8 cores handles 4 batches
([4, 512, 64, 64] shard, 32 MiB). No collectives; the host concatenates the
8 per-core [4, 1024] outputs.

Per core: 2048 rows (b*512+c) x 4096 spatial elems, streamed through SBUF
on the SP HWDGE queue as 7 m=2 slabs (32 KiB/partition descriptors, HBM-cap
rate) + 2 m=1 tail slabs (16 KiB descriptors).

v2 rebalance vs the all-DVE baseline (which had DVE busy ~92 us ~= the whole
stream time, so on slow-HBM runs bn_stats lagged the last DMA byte by 15+ us):
  - ACT consumes m=2 slabs 1 and 3 via Copy/Square+accumulate and also row 1
    of slab 6 (the last m=2 slab, which lands ~12 us before stream end —
    row-splitting it across DVE+ACT halves its post-arrival latency).
  - DVE consumes slabs 0, 2, 4, 5, row 0 of slab 6, and the two m=1 tails,
    ~68 us of bn_stats/bn_aggr against a ~95-100 us stream: enough slack
    that DVE is always idle-ready when the tail slabs land.
  - ACT's one-time ACT_TABLE_LOAD (~1.5 us) is triggered by a dummy copy at
    block start instead of inside the first epilogue.
  - The final F->T transpose is gated on exactly the tail epilogues (the
    only F writers), not on every slab's epilogue.

Outputs: m=2 slabs DMA their [128, 2, 2] stats directly (interleaved channel
layout, 3D AP; the ~7 us completion latency of the tiny per-partition
descriptors hides mid-stream). The two m=1 tails write stat columns of
F[128, 4]; DVE block-transposes F -> T so the one final DMA writes 4
contiguous 512 B runs.

Raw Bass with manual semaphores; every cross-instruction data edge is
covered by an explicit semaphore observation for the CoreSim race detector.
SWDGE requires sem values to start at 0, so semaphores are never reused.
"""

from contextlib import ExitStack

import numpy as np

B, C, H, W = 32, 512, 64, 64
N_CORES = 8
B_LOCAL = B // N_CORES  # 4
N = H * W  # 4096
P = 128
ROWS = B_LOCAL * C  # 2048

NBUF = 5  # xt ring slots (32 KiB/partition each)
NRES = 4  # res ring slots / out-DMA sems

M2SLABS = 7  # m=2 slabs, rows 0..1791
ACT_FULL = (1, 3)  # m=2 slabs fully consumed by ACT accumulate
SPLIT = 6  # last m=2 slab: DVE row 0, ACT row 1
DVE_FULL = (0, 2, 4, 5)
TAILROWS = 2  # m=1 tail slabs, rows 1792..2047
# tail row 1 column split: A-part groups (8 bn_stats over 3968 cols) + one
# trailing 128-col B-part (512 B/partition descriptors, the line-rate floor)
T1GROUPS = [512] * 7 + [384]
BCOL = sum(T1GROUPS)  # 3968

_CACHE = {}


def _build():
    import concourse.bass as bass
    from concourse import mybir

    nc = bass.Bass()
    x_ext = nc.declare_dram_parameter(
        "f_vol", [B_LOCAL, C, H, W], mybir.dt.float32, isOutput=False
    )
    out_ext = nc.declare_dram_parameter(
        "out", [B_LOCAL, 2 * C], mybir.dt.float32, isOutput=True
    )

    x = x_ext.ap().rearrange("b c h w -> (b c) (h w)")  # [2048, 4096]

    base_rows = [256 * j for j in range(M2SLABS)]
    tail_base = 256 * M2SLABS  # 1792
    tb, tc0 = divmod(tail_base, C)  # batch 3, channel 256
    for j in range(M2SLABS):
        assert (base_rows[j] % C) + 256 <= C, f"slab {j} crosses a batch"
    assert tc0 + TAILROWS * P <= C

    # --- semaphore value accounting (matches emission order below) ---
    # dve_stats: +1 per bn_stats. DVE order: slabs 0,2,4,5 (16 each),
    # split-row0 (8), tail0 (8), tail1 (8).
    dve_cum = {0: 16, 2: 32, 4: 48, 5: 64, "s6r0": 72, "t0": 80, "t1": 89}
    # mv_ready: +1 per bn_aggr, same order (2 per full slab, 1 each after).
    mv_after = {0: 2, 2: 4, 4: 6, 5: 8, "s6r0": 9, "t0": 10, "t1": 11}
    # act_stats: +1 per ACT accumulate pass: slab1 (4), slab3 (4), s6r1 (2).
    acts_after = {1: 4, 3: 8, "s6r1": 10}

    # res ring over the 7 per-slab out-DMAs
    rslot = {j: j % NRES for j in range(M2SLABS)}
    out_total = {s: 0 for s in range(NRES)}
    res_wait = {}
    for j in range(M2SLABS):
        s = rslot[j]
        res_wait[j] = out_total[s]
        out_total[s] += 16

    # act_done: +1 per ACT epilogue op, in ACT emission order:
    # epi0 (4), epi1 (8 acc-form ops), epi2, epi3, epi4, epi5, epi6
    # (2 mv-form + 4 acc-form), tail epis (2 each).
    actd_after = {}
    cact = 0
    for j in range(M2SLABS):
        if j in ACT_FULL:
            cact += 8
        elif j == SPLIT:
            cact += 6
        else:
            cact += 4
        actd_after[j] = cact
    for t in range(TAILROWS):
        cact += 2
        actd_after[("tail", t)] = cact
    ACT_TOTAL = cact

    with ExitStack() as ctx:
        block = ctx.enter_context(nc.Block(no_gpsimd_drain=True))
        dma_m2 = [
            ctx.enter_context(nc.semaphore(f"dma_m2_{j}")) for j in range(M2SLABS)
        ]
        dma_t = [ctx.enter_context(nc.semaphore(f"dma_t{t}")) for t in range(2)]
        dma_t1b = ctx.enter_context(nc.semaphore("dma_t1b"))
        dma_out = [ctx.enter_context(nc.semaphore(f"dma_out{s}")) for s in range(NRES)]
        dma_fin = ctx.enter_context(nc.semaphore("dma_fin"))
        dve_stats = ctx.enter_context(nc.semaphore("dve_stats"))
        act_stats = ctx.enter_context(nc.semaphore("act_stats"))
        mv_ready = ctx.enter_context(nc.semaphore("mv_ready"))
        act_done = ctx.enter_context(nc.semaphore("act_done"))
        trans_done = ctx.enter_context(nc.semaphore("trans_done"))

        xt = ctx.enter_context(nc.sbuf_tensor("xt", [P, NBUF, 2 * N], mybir.dt.float32))
        xt2 = ctx.enter_context(nc.sbuf_tensor("xt2", [P, 2, N], mybir.dt.float32))
        # stats slots: 0..3 = full DVE slabs 0,2,4,5; statsT for s6r0+tails
        stats = ctx.enter_context(
            nc.sbuf_tensor("stats", [P, 4, 2, 8, 6], mybir.dt.float32)
        )
        statsT = ctx.enter_context(
            nc.sbuf_tensor("statsT", [P, 3, 9, 6], mybir.dt.float32)
        )
        mv = ctx.enter_context(nc.sbuf_tensor("mv", [P, 4, 2, 2], mybir.dt.float32))
        mvT = ctx.enter_context(nc.sbuf_tensor("mvT", [P, 3, 2], mybir.dt.float32))
        res = ctx.enter_context(nc.sbuf_tensor("res", [P, NRES, 2, 2], mybir.dt.float32))
        F = ctx.enter_context(nc.sbuf_tensor("F", [P, 32], mybir.dt.float32))
        T = ctx.enter_context(nc.sbuf_tensor("T", [P, P], mybir.dt.float32))
        # ACT accumulators [sum, sumsq, tmp]: rows (1,r0),(1,r1),(3,r0),(3,r1),(6,r1)
        acc = ctx.enter_context(nc.sbuf_tensor("acc", [P, 5, 3], mybir.dt.float32))
        warm = ctx.enter_context(nc.sbuf_tensor("warm", [P, 2], mybir.dt.float32))

        sslot = {0: 0, 2: 1, 4: 2, 5: 3}  # stats/mv slot per full DVE slab
        accrow = {(1, 0): 0, (1, 1): 1, (3, 0): 2, (3, 1): 3, (6, 1): 4}

        @block.sync
        def _(sync):
            for j in range(M2SLABS):
                if j >= NBUF:
                    jp = j - NBUF
                    if jp in ACT_FULL:
                        sync.wait_ge(act_stats, acts_after[jp])
                    else:
                        sync.wait_ge(dve_stats, dve_cum[jp])
                    sync.wait_ge(dma_m2[jp], 16)
                src = x[base_rows[j] : base_rows[j] + 256, :].rearrange(
                    "(p m) f -> p (m f)", m=2
                )
                sync.dma_start(out=xt[:, j % NBUF, :], in_=src).then_inc(
                    dma_m2[j], 16
                )
            # tail 0: one m=1 slab. tail 1 (the last-arriving data) is split
            # by column so only a 512-col B-part trails the stream — the
            # exposed tail compute is then one bn_stats instead of eight.
            src = x[tail_base : tail_base + P, :]
            sync.dma_start(out=xt2[:, 0, :], in_=src).then_inc(dma_t[0], 16)
            srcA = x[tail_base + P : tail_base + 2 * P, 0:BCOL]
            sync.dma_start(out=xt2[:, 1, 0:BCOL], in_=srcA).then_inc(dma_t[1], 16)
            srcB = x[tail_base + P : tail_base + 2 * P, BCOL:N]
            sync.dma_start(out=xt2[:, 1, BCOL:N], in_=srcB).then_inc(dma_t1b, 16)

            # final output DMA: T rows 0..3 -> 4 contiguous 512 B runs
            sync.wait_ge(trans_done, 2 + P // 32)
            dst = bass.AP(
                tensor=out_ext,
                offset=tb * 2 * C + tc0,
                ap=[[P, TAILROWS], [C, 2], [1, P]],
            )
            sync.dma_start(out=dst, in_=T[0 : 2 * TAILROWS, 0:P]).then_inc(
                dma_fin, 16
            )
            for s in range(NRES):
                if out_total[s]:
                    sync.wait_ge(dma_out[s], out_total[s])
            sync.wait_ge(dma_fin, 16)

        @block.vector
        def _(vector):
            # trans_done: 1 = warm memset, 2 = F memset, 3..6 = transposes
            vector.memset(warm[:, :], 0.0).then_inc(trans_done, 1)
            vector.memset(F[:, :], 0.0).then_inc(trans_done, 1)

            for j in DVE_FULL:
                s = sslot[j]
                vector.wait_ge(dma_m2[j], 16)
                for r in range(2):
                    for g in range(8):
                        vector.bn_stats(
                            out=stats[:, s, r, g, :],
                            in_=xt[:, j % NBUF, (r * 8 + g) * 512 : (r * 8 + g + 1) * 512],
                        ).then_inc(dve_stats, 1)
                vector.wait_ge(dve_stats, dve_cum[j])
                for r in range(2):
                    vector.bn_aggr(
                        out=mv[:, s, r, :], in_=stats[:, s, r, :, :]
                    ).then_inc(mv_ready, 1)

            # split slab: row 0 only (row 1 is ACT's)
            vector.wait_ge(dma_m2[SPLIT], 16)
            for g in range(8):
                vector.bn_stats(
                    out=statsT[:, 0, g, :],
                    in_=xt[:, SPLIT % NBUF, g * 512 : (g + 1) * 512],
                ).then_inc(dve_stats, 1)
            vector.wait_ge(dve_stats, dve_cum["s6r0"])
            vector.bn_aggr(out=mvT[:, 0, :], in_=statsT[:, 0, 0:8, :]).then_inc(
                mv_ready, 1
            )

            for t in range(TAILROWS):
                vector.wait_ge(dma_t[t], 16)
                if t == 0:
                    for g in range(8):
                        vector.bn_stats(
                            out=statsT[:, 1, g, :],
                            in_=xt2[:, 0, g * 512 : (g + 1) * 512],
                        ).then_inc(dve_stats, 1)
                else:
                    c0 = 0
                    for g, gw in enumerate(T1GROUPS):
                        vector.bn_stats(
                            out=statsT[:, 2, g, :], in_=xt2[:, 1, c0 : c0 + gw]
                        ).then_inc(dve_stats, 1)
                        c0 += gw
                    assert c0 == BCOL
                    vector.wait_ge(dma_t1b, 16)
                    vector.bn_stats(
                        out=statsT[:, 2, len(T1GROUPS), :], in_=xt2[:, 1, BCOL:N]
                    ).then_inc(dve_stats, 1)
                vector.wait_ge(dve_stats, dve_cum[f"t{t}"])
                vector.bn_aggr(
                    out=mvT[:, 1 + t, :],
                    in_=statsT[:, 1 + t, 0 : (8 if t == 0 else len(T1GROUPS) + 1), :],
                ).then_inc(mv_ready, 1)

            # all F writers done -> 32x32 block transposes of F into T
            vector.wait_ge(act_done, ACT_TOTAL)
            vector.wait_ge(trans_done, 2)  # observe the F memset (same engine)
            for blk in range(P // 32):
                vector.transpose(
                    out=T[0:32, blk * 32 : blk * 32 + 32],
                    in_=F[blk * 32 : blk * 32 + 32, 0:32],
                ).then_inc(trans_done, 1)

        @block.scalar
        def _(scalar):
            A = 1.0 / np.sqrt(float(N) * (N - 1))
            cact = 0
            casts = 0

            # warm the ACT function table while the first slab streams
            scalar.wait_ge(trans_done, 1)
            scalar.activation(
                out=warm[:, 0:1],
                in_=warm[:, 1:2],
                func=mybir.ActivationFunctionType.Copy,
            )

            def acc_pass(j, rows):
                nonlocal casts
                scalar.wait_ge(dma_m2[j], 16)
                for r in rows:
                    row = xt[:, j % NBUF, r * N : (r + 1) * N]
                    ar = accrow[(j, r)]
                    scalar.activation(
                        out=row,
                        in_=row,
                        func=mybir.ActivationFunctionType.Copy,
                        accum_out=acc[:, ar, 0:1],
                    ).then_inc(act_stats, 1)
                    casts += 1
                    # observe the Copy (xt write + acc[0]) before Square
                    scalar.wait_ge(act_stats, casts)
                    scalar.activation(
                        out=row,
                        in_=row,
                        func=mybir.ActivationFunctionType.Square,
                        accum_out=acc[:, ar, 1:2],
                    ).then_inc(act_stats, 1)
                    casts += 1

            def out_dma(j):
                b, c0 = divmod(base_rows[j], C)
                s = rslot[j]
                dst = bass.AP(
                    tensor=out_ext,
                    offset=b * 2 * C + c0,
                    ap=[[2, P], [C, 2], [1, 2]],
                )
                scalar.dma_start(out=dst, in_=res[:, s, :, :]).then_inc(
                    dma_out[s], 16
                )

            def mv_stat_ops(mean_src, var_src, mean_dst, std_dst):
                nonlocal cact
                scalar.copy(out=mean_dst, in_=mean_src).then_inc(act_done, 1)
                scalar.activation(
                    out=std_dst,
                    in_=var_src,
                    func=mybir.ActivationFunctionType.Sqrt,
                    scale=float(N) / (N - 1),
                ).then_inc(act_done, 1)
                cact += 2

            def acc_stat_ops(ar, mean_dst, std_dst):
                nonlocal cact
                # mean = sum / N
                scalar.activation(
                    out=mean_dst,
                    in_=acc[:, ar, 0:1],
                    func=mybir.ActivationFunctionType.Copy,
                    scale=1.0 / N,
                ).then_inc(act_done, 1)
                # tmp = (sum*A)^2 = sum^2/(N(N-1))
                scalar.activation(
                    out=acc[:, ar, 2:3],
                    in_=acc[:, ar, 0:1],
                    func=mybir.ActivationFunctionType.Square,
                    scale=A,
                ).then_inc(act_done, 1)
                cact += 2
                scalar.wait_ge(act_done, cact)
                scalar.activation(
                    out=acc[:, ar, 2:3],
                    in_=acc[:, ar, 2:3],
                    func=mybir.ActivationFunctionType.Copy,
                    scale=-1.0,
                ).then_inc(act_done, 1)
                cact += 1
                scalar.wait_ge(act_done, cact)
                # std = sqrt(sumsq/(N-1) - sum^2/(N(N-1)))
                scalar.activation(
                    out=std_dst,
                    in_=acc[:, ar, 1:2],
                    func=mybir.ActivationFunctionType.Sqrt,
                    scale=1.0 / (N - 1),
                    bias=acc[:, ar, 2:3],
                ).then_inc(act_done, 1)
                cact += 1

            def epilogue(j):
                s = rslot[j]
                if j in ACT_FULL:
                    scalar.wait_ge(act_stats, acts_after[j])
                elif j == SPLIT:
                    scalar.wait_ge(mv_ready, mv_after["s6r0"])
                    scalar.wait_ge(act_stats, acts_after["s6r1"])
                else:
                    scalar.wait_ge(mv_ready, mv_after[j])
                if res_wait[j]:
                    scalar.wait_ge(dma_out[s], res_wait[j])
                for r in range(2):
                    mean_dst = res[:, s, 0, r : r + 1]
                    std_dst = res[:, s, 1, r : r + 1]
                    if j in ACT_FULL:
                        acc_stat_ops(accrow[(j, r)], mean_dst, std_dst)
                    elif j == SPLIT and r == 1:
                        acc_stat_ops(accrow[(j, 1)], mean_dst, std_dst)
                    elif j == SPLIT:
                        mv_stat_ops(
                            mvT[:, 0, 0:1], mvT[:, 0, 1:2], mean_dst, std_dst
                        )
                    else:
                        mv_stat_ops(
                            mv[:, sslot[j], r, 0:1],
                            mv[:, sslot[j], r, 1:2],
                            mean_dst,
                            std_dst,
                        )
                scalar.wait_ge(act_done, cact)  # res RAW before out DMA
                out_dma(j)
                assert cact == actd_after[j], (j, cact, actd_after[j])

            acc_pass(1, (0, 1))
            epilogue(0)
            epilogue(1)
            acc_pass(3, (0, 1))
            epilogue(2)
            epilogue(3)
            epilogue(4)
            epilogue(5)
            acc_pass(SPLIT, (1,))
            epilogue(SPLIT)

            # tail epilogues into F — kept last so ACT is idle-ready
            scalar.wait_ge(trans_done, 2)  # F memset observed
            for t in range(TAILROWS):
                scalar.wait_ge(mv_ready, mv_after[f"t{t}"])
                mv_stat_ops(
                    mvT[:, 1 + t, 0:1],
                    mvT[:, 1 + t, 1:2],
                    F[:, 2 * t : 2 * t + 1],
                    F[:, 2 * t + 1 : 2 * t + 2],
                )
                assert cact == actd_after[("tail", t)]
            assert cact == ACT_TOTAL, (cact, ACT_TOTAL)

    return nc


def kernel(f_vol: np.ndarray) -> np.ndarray:
    from concourse.bass_utils import run_bass_kernel_spmd

    if "nc" not in _CACHE:
        _CACHE["nc"] = _build()
    nc = _CACHE["nc"]

    f_vol = np.ascontiguousarray(f_vol, dtype=np.float32)
    in_maps = [
        {"f_vol": f_vol[i * B_LOCAL : (i + 1) * B_LOCAL]} for i in range(N_CORES)
    ]
    res = run_bass_kernel_spmd(nc, in_maps, core_ids=list(range(N_CORES)))
    return np.concatenate([res.results[i]["out"] for i in range(N_CORES)], axis=0)
